# revision 11
# baseline (speedup 1.0000x reference)
"""Chamfer (AutoEncoder) loss on 8 Trainium2 NeuronCores.

Problem: predictions [16, 2048, 3], targets [16, 2048, 3] (float32).
loss = sum_b [ sum_i min_j ||x_bi - y_bj||^2 + sum_j min_i ||x_bi - y_bj||^2 ]

Strategy
--------
Data-parallel over the batch: 16 batches / 8 cores = 2 per core. Each
(batch, direction) pair is one of 4 identical "jobs" per core.

The pairwise squared-distance matrix is produced directly by K-stacked
matmuls via the augmentation trick (negated so the device computes -P
and all minima become maxima):
    a' = -[a0, a1, a2, |a|^2, 1]           (5 x n)
    b' =  [-2*b0, -2*b1, -2*b2, 1, |b|^2]  (5 x m)
    a'.T @ b' = -P,  P[i, j] = |a_i|^2 + |b_j|^2 - 2 a_i.b_j

fp32 matmul on TRN2 runs in LOW_HIGH mode (~8x slower than bf16), so
operands are split hi/lo in bf16 and the three product terms
(hi*hi + hi*lo + lo*hi) are stacked along the contraction dim:
    lhsT = [a'_hi; a'_hi; a'_lo]  (15 x n, bf16)
    rhs  = [b'_hi; b'_lo; b'_hi]  (15 x m, bf16)
One K=15 bf16 matmul per output tile then yields -P at ~fp32 precision
(PE time scales with output columns, not K; PSUM accumulates in fp32;
the dropped lo*lo term is O(2^-17) relative).

PE row-group rotation: the K=15 weights occupy one 32-row group of the
128x128 array. Operands are replicated at partition offsets 0/32/64 and
consecutive output tiles rotate across those three row groups, so three
sub-array pipelines run concurrently (~3x matmul issue rate) and each
LDWEIGHTS overlaps other groups' in-flight matmuls.

Each job: 16 row-strips of [128, 2048] built by 4 matmuls (one PSUM bank
each). The strip is drained by two engines in parallel: ACT copies the
upper half to SBUF while a single custom-DVE MAX2_REDUCE instruction
consumes the PSUM lower half paired with that SBUF copy (2 elements per
DVE cycle), max-folding into one column of a [128, 16] accumulator.
Per-core output is [4, 128, 16] of -min; the host sums and negates
(the final scalar all-reduce) and returns the float32 scalar.
"""

import ml_dtypes
import numpy as np

import concourse.mybir as mybir
import concourse.tile as tile
from concourse import bacc
from concourse.bass_utils import run_bass_kernel_spmd

from max2_op import MAX2_REDUCE

B, N, M, D = 16, 2048, 2048, 3
N_CORES = 8
BPC = B // N_CORES  # batches per core
JOBS = 2 * BPC  # (batch, direction) pairs per core
ROW_TILES = N // 128  # 16
COL_CHUNK = 512
COL_CHUNKS = M // COL_CHUNK  # 4
KCAT = 15  # [hi; hi; lo] x [hi; lo; hi]

_F32 = mybir.dt.float32
_BF16 = mybir.dt.bfloat16
_NP_BF16 = ml_dtypes.bfloat16

_cached_nc = None


def _build_nc():
    nc = bacc.Bacc("TRN2", target_bir_lowering=False, debug=False)
    lhs = nc.dram_tensor("lhs", [JOBS, 3, KCAT, N], _BF16, kind="ExternalInput")
    rhs = nc.dram_tensor("rhs", [JOBS, 3, KCAT, M], _BF16, kind="ExternalInput")
    out = nc.dram_tensor("maxs", [JOBS, 128, ROW_TILES], _F32, kind="ExternalOutput")

    with tile.TileContext(nc) as tc:
        with (
            tc.tile_pool(name="inp", bufs=2) as inp_pool,
            tc.tile_pool(name="psum", bufs=2, space="PSUM") as psum_pool,
            tc.tile_pool(name="acc", bufs=2) as acc_pool,
            tc.tile_pool(name="upper", bufs=2) as upper_pool,
        ):
            for j in range(JOBS):
                # Operands replicated at partition offsets 0/32/64 for PE
                # row-group rotation.
                lhs_sb = inp_pool.tile([128, N], _BF16, tag="lhs")
                rhs_sb = inp_pool.tile([128, M], _BF16, tag="rhs")
                # Replicas at partition offsets 0/32/64 for PE row-group
                # rotation. Job 0's loads gate the pipeline ramp, so spread
                # them across three engines' DMA queues; later jobs' loads
                # hide under compute on the sync queue.
                engines = (nc.sync, nc.scalar, nc.gpsimd) if j == 0 else (nc.sync,) * 3
                for a, g in enumerate((0, 32, 64)):
                    engines[a].dma_start(lhs_sb[g : g + KCAT, :], lhs[j, a])
                    engines[a].dma_start(rhs_sb[g : g + KCAT, :], rhs[j, a])
                maxs_sb = acc_pool.tile([128, ROW_TILES], _F32, tag="maxs")
                dummy = acc_pool.tile([128, 1], _F32, tag="dummy")
                for i in range(ROW_TILES):
                    # Separate PSUM tiles for the ACT-drained upper half and
                    # the DVE-drained lower half so each recycles as soon as
                    # its own reader finishes (deeper pipeline than one
                    # monolithic 4-bank strip).
                    hi_ps = psum_pool.tile([128, M // 2], _F32, tag="hi")
                    lo_ps = psum_pool.tile([128, M // 2], _F32, tag="lo")
                    li = slice(i * 128, (i + 1) * 128)
                    # Upper-half banks first so the ACT copy can start while
                    # PE fills the lower half; chunks rotate row groups.
                    for k, (dst, half) in enumerate(
                        ((hi_ps, 0), (hi_ps, 1), (lo_ps, 0), (lo_ps, 1))
                    ):
                        g = ((i * 4 + k) % 3) * 32
                        c = 2 + k if k < 2 else k - 2
                        cs = slice(c * COL_CHUNK, (c + 1) * COL_CHUNK)
                        nc.tensor.matmul(
                            dst[:, half * COL_CHUNK : (half + 1) * COL_CHUNK],
                            lhs_sb[g : g + KCAT, li],
                            rhs_sb[g : g + KCAT, cs],
                            start=True,
                            stop=True,
                        )
                        if k == 1:
                            upper = upper_pool.tile([128, M // 2], _F32, tag="upper")
                            nc.scalar.copy(upper[:], hi_ps[:])
                    # One DVE instruction drains the PSUM lower half paired
                    # with the SBUF upper copy: accum = max over the strip.
                    nc.vector._custom_dve(
                        MAX2_REDUCE,
                        out=dummy.broadcast_to((128, M // 2)),
                        in0=lo_ps[:],
                        in1=upper[:],
                        accum_out=maxs_sb[:, i : i + 1],
                    )
                nc.sync.dma_start(out[j], maxs_sb[:])
    nc.compile()
    return nc


def _get_nc():
    global _cached_nc
    if _cached_nc is None:
        _cached_nc = _build_nc()
    return _cached_nc


def _augment(a, b):
    """a: [n, 3], b: [m, 3] -> (lhsT [5, n], rhs [5, m]) float32.

    lhsT is negated so the device matmul yields -P.
    """
    n = a.shape[0]
    m = b.shape[0]
    lhsT = np.empty((5, n), dtype=np.float32)
    lhsT[0:3] = -a.T
    lhsT[3] = -(a * a).sum(axis=1)
    lhsT[4] = -1.0
    rhs = np.empty((5, m), dtype=np.float32)
    rhs[0:3] = -2.0 * b.T
    rhs[3] = 1.0
    rhs[4] = (b * b).sum(axis=1)
    return lhsT, rhs


def _split_cat(lhs, rhs):
    """fp32 [J, 5, n] operands -> K-stacked bf16 [J, 15, n] hi/lo forms."""
    lh = lhs.astype(_NP_BF16)
    ll = (lhs - lh.astype(np.float32)).astype(_NP_BF16)
    rh = rhs.astype(_NP_BF16)
    rl = (rhs - rh.astype(np.float32)).astype(_NP_BF16)
    lcat = np.concatenate([lh, lh, ll], axis=1)
    rcat = np.concatenate([rh, rl, rh], axis=1)
    # Replicate for the three PE row groups (partition offsets 0/32/64).
    lrep = np.repeat(lcat[:, None, :, :], 3, axis=1)
    rrep = np.repeat(rcat[:, None, :, :], 3, axis=1)
    return np.ascontiguousarray(lrep), np.ascontiguousarray(rrep)


def _in_maps(predictions, targets):
    in_maps = []
    for core in range(N_CORES):
        lhs = np.empty((JOBS, 5, N), dtype=np.float32)
        rhs = np.empty((JOBS, 5, M), dtype=np.float32)
        for bi in range(BPC):
            b = core * BPC + bi
            # direction 0: rows = predictions, min over targets
            lhs[2 * bi], rhs[2 * bi] = _augment(predictions[b], targets[b])
            # direction 1: rows = targets, min over predictions
            lhs[2 * bi + 1], rhs[2 * bi + 1] = _augment(targets[b], predictions[b])
        lcat, rcat = _split_cat(lhs, rhs)
        in_maps.append({"lhs": lcat, "rhs": rcat})
    return in_maps


def kernel(predictions, targets):
    predictions = np.asarray(predictions, dtype=np.float32)
    targets = np.asarray(targets, dtype=np.float32)

    nc = _get_nc()
    res = run_bass_kernel_spmd(
        nc, _in_maps(predictions, targets), core_ids=list(range(N_CORES))
    )

    total = 0.0
    for core in range(N_CORES):
        total -= res.results[core]["maxs"].astype(np.float64).sum()
    return np.float32(total)


# revision 14
# speedup vs baseline: 1.1394x; 1.1394x over previous
"""Chamfer (AutoEncoder) loss on 8 Trainium2 NeuronCores.

Problem: predictions [16, 2048, 3], targets [16, 2048, 3] (float32).
loss = sum_b [ sum_i min_j ||x_bi - y_bj||^2 + sum_j min_i ||x_bi - y_bj||^2 ]

Strategy
--------
Data-parallel over the batch: 16 batches / 8 cores = 2 per core. Each
(batch, direction) pair is one of 4 identical "jobs" per core.

The pairwise squared-distance matrix is produced directly by K-stacked
matmuls via the augmentation trick (negated so the device computes -P
and all minima become maxima):
    a' = -[a0, a1, a2, |a|^2, 1]           (5 x n)
    b' =  [-2*b0, -2*b1, -2*b2, 1, |b|^2]  (5 x m)
    a'.T @ b' = -P,  P[i, j] = |a_i|^2 + |b_j|^2 - 2 a_i.b_j

fp32 matmul on TRN2 runs in LOW_HIGH mode (~8x slower than bf16), so
operands are split hi/lo in bf16 and the three product terms
(hi*hi + hi*lo + lo*hi) are stacked along the contraction dim:
    lhsT = [a'_hi; a'_hi; a'_lo]  (15 x n, bf16)
    rhs  = [b'_hi; b'_lo; b'_hi]  (15 x m, bf16)
One K=15 bf16 matmul per output tile then yields -P at ~fp32 precision
(PE time scales with output columns, not K; PSUM accumulates in fp32;
the dropped lo*lo term is O(2^-17) relative).

PE row-group rotation: the K=15 weights occupy one 32-row group of the
128x128 array. Operands are replicated at partition offsets 0/32/64 and
consecutive output tiles rotate across those three row groups, so three
sub-array pipelines run concurrently (~3x matmul issue rate) and each
LDWEIGHTS overlaps other groups' in-flight matmuls.

Each job: 16 row-strips of [128, 2048] built by 4 matmuls (one PSUM bank
each). The strip is drained by two engines in parallel: ACT copies the
upper half to SBUF while a single custom-DVE MAX2_REDUCE instruction
consumes the PSUM lower half paired with that SBUF copy (2 elements per
DVE cycle), max-folding into one column of a [128, 16] accumulator.
Per-core output is [4, 128, 16] of -min; the host sums and negates
(the final scalar all-reduce) and returns the float32 scalar.
"""

import ml_dtypes
import numpy as np

import concourse.dve_ops as dve_ops
import concourse.mybir as mybir
import concourse.tile as tile
from concourse import bacc
from concourse.bass_utils import run_bass_kernel_spmd
from concourse.dve_ops import DveOp
from concourse.dve_spec import Spec, Src0, Src1, _has_src1, lower, maxx
from concourse.dve_table_gen import dve_ver_for  # noqa: F401  (ver sanity)
from concourse.dve_uop import DveOpSpec


def _register_max2() -> DveOp:
    """Custom DVE op: body = max(Src0, Src1), accum_out = max fold.

    Consumes two tensor streams per cycle (one may be PSUM); with negated
    inputs this is a paired min-reduction. Registered into the live
    concourse.dve_ops tables (the per-NEFF DVE table generator resolves
    ops by name from dve_ops.OPS).
    """
    for existing in dve_ops.OPS:
        if existing.name == "MAX2_REDUCE_ANT":
            return existing
    spec = Spec(
        body=maxx(Src0, Src1),
        accum=maxx,
        reference=lambda in0, in1, s0, s1, imm2: (
            np.maximum(in0.astype(np.float32), in1.astype(np.float32)),
            np.maximum.reduce(
                np.maximum(in0.astype(np.float32), in1.astype(np.float32)),
                axis=tuple(range(1, in0.ndim)),
            ).reshape(in0.shape[0], 1),
        ),
    )
    name = "MAX2_REDUCE_ANT"
    row = dve_ops._CUSTOM_DVE_ROW_BASE + len(dve_ops.OPS)
    shas = {}
    for ver in ("v3", "v4"):
        try:
            uops = lower(spec, ver=ver)
        except Exception:
            continue
        shas[ver] = DveOpSpec(
            name=name, opcode=row, uops=uops, rd1_en=_has_src1(spec)
        ).sha(ver)
    op = DveOp(name, spec, subdim=False, uops_sha=shas)
    dve_ops.OPS.append(op)
    dve_ops._SUB_OPCODE_FOR_NAME[op.name] = row
    dve_ops.CUSTOM_DVE_SPECS[op.name] = op.spec
    assert max(dve_ops._SUB_OPCODE_FOR_NAME.values()) < 0x20
    return op


MAX2_REDUCE = _register_max2()

B, N, M, D = 16, 2048, 2048, 3
N_CORES = 8
BPC = B // N_CORES  # batches per core
JOBS = 2 * BPC  # (batch, direction) pairs per core
ROW_TILES = N // 128  # 16
COL_CHUNK = 512
COL_CHUNKS = M // COL_CHUNK  # 4
KCAT = 15  # [hi; hi; lo] x [hi; lo; hi]

_F32 = mybir.dt.float32
_BF16 = mybir.dt.bfloat16
_NP_BF16 = ml_dtypes.bfloat16

_cached_nc = None


def _build_nc():
    nc = bacc.Bacc("TRN2", target_bir_lowering=False, debug=False)
    lhs = nc.dram_tensor("lhs", [JOBS, 3, KCAT, N], _BF16, kind="ExternalInput")
    rhs = nc.dram_tensor("rhs", [JOBS, 3, KCAT, M], _BF16, kind="ExternalInput")
    out = nc.dram_tensor("maxs", [JOBS, 128, ROW_TILES], _F32, kind="ExternalOutput")

    with tile.TileContext(nc) as tc:
        with (
            tc.tile_pool(name="inp", bufs=2) as inp_pool,
            tc.tile_pool(name="psum", bufs=2, space="PSUM") as psum_pool,
            tc.tile_pool(name="acc", bufs=2) as acc_pool,
            tc.tile_pool(name="upper", bufs=4) as upper_pool,
        ):
            for j in range(JOBS):
                # Operands replicated at partition offsets 0/32/64 for PE
                # row-group rotation.
                lhs_sb = inp_pool.tile([128, N], _BF16, tag="lhs")
                rhs_sb = inp_pool.tile([128, M], _BF16, tag="rhs")
                # Replicas at partition offsets 0/32/64 for PE row-group
                # rotation. Job 0's loads gate the pipeline ramp, so spread
                # them across three engines' DMA queues; later jobs' loads
                # hide under compute on the sync queue.
                engines = (nc.sync, nc.scalar, nc.gpsimd) if j == 0 else (nc.sync,) * 3
                for a, g in enumerate((0, 32, 64)):
                    engines[a].dma_start(lhs_sb[g : g + KCAT, :], lhs[j, a])
                    engines[a].dma_start(rhs_sb[g : g + KCAT, :], rhs[j, a])
                maxs_sb = acc_pool.tile([128, ROW_TILES], _F32, tag="maxs")
                for i in range(ROW_TILES):
                    dummy = upper_pool.tile([128, 1], _F32, tag="dummy")
                    # Separate PSUM tiles for the ACT-drained upper half and
                    # the DVE-drained lower half so each recycles as soon as
                    # its own reader finishes (deeper pipeline than one
                    # monolithic 4-bank strip).
                    hi_ps = psum_pool.tile([128, M // 2], _F32, tag="hi")
                    lo_ps = psum_pool.tile([128, M // 2], _F32, tag="lo")
                    li = slice(i * 128, (i + 1) * 128)
                    # Upper-half banks first so the ACT copy can start while
                    # PE fills the lower half; chunks rotate row groups.
                    for k, (dst, half) in enumerate(
                        ((hi_ps, 0), (hi_ps, 1), (lo_ps, 0), (lo_ps, 1))
                    ):
                        g = ((i * 4 + k) % 3) * 32
                        c = 2 + k if k < 2 else k - 2
                        cs = slice(c * COL_CHUNK, (c + 1) * COL_CHUNK)
                        nc.tensor.matmul(
                            dst[:, half * COL_CHUNK : (half + 1) * COL_CHUNK],
                            lhs_sb[g : g + KCAT, li],
                            rhs_sb[g : g + KCAT, cs],
                            start=True,
                            stop=True,
                        )
                        if k == 1:
                            upper = upper_pool.tile([128, M // 2], _F32, tag="upper")
                            nc.scalar.copy(upper[:], hi_ps[:])
                    # One DVE instruction drains the PSUM lower half paired
                    # with the SBUF upper copy: accum = max over the strip.
                    nc.vector._custom_dve(
                        MAX2_REDUCE,
                        out=dummy.broadcast_to((128, M // 2)),
                        in0=lo_ps[:],
                        in1=upper[:],
                        accum_out=maxs_sb[:, i : i + 1],
                    )
                nc.sync.dma_start(out[j], maxs_sb[:])
    nc.compile()
    return nc


def _get_nc():
    global _cached_nc
    if _cached_nc is None:
        _cached_nc = _build_nc()
    return _cached_nc


def _augment(a, b):
    """a: [n, 3], b: [m, 3] -> (lhsT [5, n], rhs [5, m]) float32.

    lhsT is negated so the device matmul yields -P.
    """
    n = a.shape[0]
    m = b.shape[0]
    lhsT = np.empty((5, n), dtype=np.float32)
    lhsT[0:3] = -a.T
    lhsT[3] = -(a * a).sum(axis=1)
    lhsT[4] = -1.0
    rhs = np.empty((5, m), dtype=np.float32)
    rhs[0:3] = -2.0 * b.T
    rhs[3] = 1.0
    rhs[4] = (b * b).sum(axis=1)
    return lhsT, rhs


def _split_cat(lhs, rhs):
    """fp32 [J, 5, n] operands -> K-stacked bf16 [J, 15, n] hi/lo forms."""
    lh = lhs.astype(_NP_BF16)
    ll = (lhs - lh.astype(np.float32)).astype(_NP_BF16)
    rh = rhs.astype(_NP_BF16)
    rl = (rhs - rh.astype(np.float32)).astype(_NP_BF16)
    lcat = np.concatenate([lh, lh, ll], axis=1)
    rcat = np.concatenate([rh, rl, rh], axis=1)
    # Replicate for the three PE row groups (partition offsets 0/32/64).
    lrep = np.repeat(lcat[:, None, :, :], 3, axis=1)
    rrep = np.repeat(rcat[:, None, :, :], 3, axis=1)
    return np.ascontiguousarray(lrep), np.ascontiguousarray(rrep)


def _in_maps(predictions, targets):
    in_maps = []
    for core in range(N_CORES):
        lhs = np.empty((JOBS, 5, N), dtype=np.float32)
        rhs = np.empty((JOBS, 5, M), dtype=np.float32)
        for bi in range(BPC):
            b = core * BPC + bi
            # direction 0: rows = predictions, min over targets
            lhs[2 * bi], rhs[2 * bi] = _augment(predictions[b], targets[b])
            # direction 1: rows = targets, min over predictions
            lhs[2 * bi + 1], rhs[2 * bi + 1] = _augment(targets[b], predictions[b])
        lcat, rcat = _split_cat(lhs, rhs)
        in_maps.append({"lhs": lcat, "rhs": rcat})
    return in_maps


def kernel(predictions, targets):
    predictions = np.asarray(predictions, dtype=np.float32)
    targets = np.asarray(targets, dtype=np.float32)

    nc = _get_nc()
    res = run_bass_kernel_spmd(
        nc, _in_maps(predictions, targets), core_ids=list(range(N_CORES))
    )

    total = 0.0
    for core in range(N_CORES):
        total -= res.results[core]["maxs"].astype(np.float64).sum()
    return np.float32(total)


# revision 15
# speedup vs baseline: 1.1523x; 1.0114x over previous
"""Chamfer (AutoEncoder) loss on 8 Trainium2 NeuronCores.

Problem: predictions [16, 2048, 3], targets [16, 2048, 3] (float32).
loss = sum_b [ sum_i min_j ||x_bi - y_bj||^2 + sum_j min_i ||x_bi - y_bj||^2 ]

Strategy
--------
Data-parallel over the batch: 16 batches / 8 cores = 2 per core. Each
(batch, direction) pair is one of 4 identical "jobs" per core.

The pairwise squared-distance matrix is produced directly by K-stacked
matmuls via the augmentation trick (negated so the device computes -P
and all minima become maxima):
    a' = -[a0, a1, a2, |a|^2, 1]           (5 x n)
    b' =  [-2*b0, -2*b1, -2*b2, 1, |b|^2]  (5 x m)
    a'.T @ b' = -P,  P[i, j] = |a_i|^2 + |b_j|^2 - 2 a_i.b_j

fp32 matmul on TRN2 runs in LOW_HIGH mode (~8x slower than bf16), so
operands are split hi/lo in bf16 and the three product terms
(hi*hi + hi*lo + lo*hi) are stacked along the contraction dim:
    lhsT = [a'_hi; a'_hi; a'_lo]  (15 x n, bf16)
    rhs  = [b'_hi; b'_lo; b'_hi]  (15 x m, bf16)
One K=15 bf16 matmul per output tile then yields -P at ~fp32 precision
(PE time scales with output columns, not K; PSUM accumulates in fp32;
the dropped lo*lo term is O(2^-17) relative).

PE row-group rotation: the K=15 weights occupy one 32-row group of the
128x128 array. Operands are replicated at partition offsets 0/32/64 and
consecutive output tiles rotate across those three row groups, so three
sub-array pipelines run concurrently (~3x matmul issue rate) and each
LDWEIGHTS overlaps other groups' in-flight matmuls.

Each job: 16 row-strips of [128, 2048] built by 4 matmuls (one PSUM bank
each). The strip is drained by two engines in parallel: ACT copies the
upper half to SBUF while a single custom-DVE MAX2_REDUCE instruction
consumes the PSUM lower half paired with that SBUF copy (2 elements per
DVE cycle), max-folding into one column of a [128, 16] accumulator.
Per-core output is [4, 128, 16] of -min; the host sums and negates
(the final scalar all-reduce) and returns the float32 scalar.
"""

import ml_dtypes
import numpy as np

import concourse.dve_ops as dve_ops
import concourse.mybir as mybir
import concourse.tile as tile
from concourse import bacc
from concourse.bass_utils import run_bass_kernel_spmd
from concourse.dve_ops import DveOp
from concourse.dve_spec import Spec, Src0, Src1, _has_src1, lower, maxx
from concourse.dve_table_gen import dve_ver_for  # noqa: F401  (ver sanity)
from concourse.dve_uop import DveOpSpec


def _register_max2() -> DveOp:
    """Custom DVE op: body = max(Src0, Src1), accum_out = max fold.

    Consumes two tensor streams per cycle (one may be PSUM); with negated
    inputs this is a paired min-reduction. Registered into the live
    concourse.dve_ops tables (the per-NEFF DVE table generator resolves
    ops by name from dve_ops.OPS).
    """
    for existing in dve_ops.OPS:
        if existing.name == "MAX2_REDUCE_ANT":
            return existing
    spec = Spec(
        body=maxx(Src0, Src1),
        accum=maxx,
        reference=lambda in0, in1, s0, s1, imm2: (
            np.maximum(in0.astype(np.float32), in1.astype(np.float32)),
            np.maximum.reduce(
                np.maximum(in0.astype(np.float32), in1.astype(np.float32)),
                axis=tuple(range(1, in0.ndim)),
            ).reshape(in0.shape[0], 1),
        ),
    )
    name = "MAX2_REDUCE_ANT"
    row = dve_ops._CUSTOM_DVE_ROW_BASE + len(dve_ops.OPS)
    shas = {}
    for ver in ("v3", "v4"):
        try:
            uops = lower(spec, ver=ver)
        except Exception:
            continue
        shas[ver] = DveOpSpec(
            name=name, opcode=row, uops=uops, rd1_en=_has_src1(spec)
        ).sha(ver)
    op = DveOp(name, spec, subdim=False, uops_sha=shas)
    dve_ops.OPS.append(op)
    dve_ops._SUB_OPCODE_FOR_NAME[op.name] = row
    dve_ops.CUSTOM_DVE_SPECS[op.name] = op.spec
    assert max(dve_ops._SUB_OPCODE_FOR_NAME.values()) < 0x20
    return op


MAX2_REDUCE = _register_max2()

B, N, M, D = 16, 2048, 2048, 3
N_CORES = 8
BPC = B // N_CORES  # batches per core
JOBS = 2 * BPC  # (batch, direction) pairs per core
ROW_TILES = N // 128  # 16
COL_CHUNK = 512
COL_CHUNKS = M // COL_CHUNK  # 4
KCAT = 15  # [hi; hi; lo] x [hi; lo; hi]

_F32 = mybir.dt.float32
_BF16 = mybir.dt.bfloat16
_NP_BF16 = ml_dtypes.bfloat16

_cached_nc = None


def _build_nc():
    nc = bacc.Bacc("TRN2", target_bir_lowering=False, debug=False)
    lhs = nc.dram_tensor("lhs", [JOBS, 3, KCAT, N], _BF16, kind="ExternalInput")
    rhs = nc.dram_tensor("rhs", [JOBS, 3, KCAT, M], _BF16, kind="ExternalInput")
    out = nc.dram_tensor("maxs", [JOBS, 128, ROW_TILES], _F32, kind="ExternalOutput")

    with tile.TileContext(nc) as tc:
        with (
            tc.tile_pool(name="inp", bufs=2) as inp_pool,
            tc.tile_pool(name="psum", bufs=2, space="PSUM") as psum_pool,
            tc.tile_pool(name="acc", bufs=2) as acc_pool,
            tc.tile_pool(name="upper", bufs=6) as upper_pool,
        ):
            for j in range(JOBS):
                # Operands replicated at partition offsets 0/32/64 for PE
                # row-group rotation.
                lhs_sb = inp_pool.tile([128, N], _BF16, tag="lhs")
                rhs_sb = inp_pool.tile([128, M], _BF16, tag="rhs")
                # Replicas at partition offsets 0/32/64 for PE row-group
                # rotation. Job 0's loads gate the pipeline ramp, so spread
                # them across three engines' DMA queues; later jobs' loads
                # hide under compute on the sync queue.
                engines = (nc.sync, nc.scalar, nc.gpsimd) if j == 0 else (nc.sync,) * 3
                for a, g in enumerate((0, 32, 64)):
                    engines[a].dma_start(lhs_sb[g : g + KCAT, :], lhs[j, a])
                    engines[a].dma_start(rhs_sb[g : g + KCAT, :], rhs[j, a])
                maxs_sb = acc_pool.tile([128, ROW_TILES], _F32, tag="maxs")
                for i in range(ROW_TILES):
                    dummy = upper_pool.tile([128, 1], _F32, tag="dummy")
                    # Separate PSUM tiles for the ACT-drained upper half and
                    # the DVE-drained lower half so each recycles as soon as
                    # its own reader finishes (deeper pipeline than one
                    # monolithic 4-bank strip).
                    hi_ps = psum_pool.tile([128, M // 2], _F32, tag="hi")
                    lo_ps = psum_pool.tile([128, M // 2], _F32, tag="lo")
                    li = slice(i * 128, (i + 1) * 128)
                    # Upper-half banks first so the ACT copy can start while
                    # PE fills the lower half; chunks rotate row groups.
                    for k, (dst, half) in enumerate(
                        ((hi_ps, 0), (hi_ps, 1), (lo_ps, 0), (lo_ps, 1))
                    ):
                        g = ((i * 4 + k) % 3) * 32
                        c = 2 + k if k < 2 else k - 2
                        cs = slice(c * COL_CHUNK, (c + 1) * COL_CHUNK)
                        nc.tensor.matmul(
                            dst[:, half * COL_CHUNK : (half + 1) * COL_CHUNK],
                            lhs_sb[g : g + KCAT, li],
                            rhs_sb[g : g + KCAT, cs],
                            start=True,
                            stop=True,
                        )
                        if k == 1:
                            upper = upper_pool.tile([128, M // 2], _F32, tag="upper")
                            nc.scalar.copy(upper[:], hi_ps[:])
                    # One DVE instruction drains the PSUM lower half paired
                    # with the SBUF upper copy: accum = max over the strip.
                    nc.vector._custom_dve(
                        MAX2_REDUCE,
                        out=dummy.broadcast_to((128, M // 2)),
                        in0=lo_ps[:],
                        in1=upper[:],
                        accum_out=maxs_sb[:, i : i + 1],
                    )
                nc.sync.dma_start(out[j], maxs_sb[:])
    nc.compile()
    return nc


def _get_nc():
    global _cached_nc
    if _cached_nc is None:
        _cached_nc = _build_nc()
    return _cached_nc


def _augment(a, b):
    """a: [n, 3], b: [m, 3] -> (lhsT [5, n], rhs [5, m]) float32.

    lhsT is negated so the device matmul yields -P.
    """
    n = a.shape[0]
    m = b.shape[0]
    lhsT = np.empty((5, n), dtype=np.float32)
    lhsT[0:3] = -a.T
    lhsT[3] = -(a * a).sum(axis=1)
    lhsT[4] = -1.0
    rhs = np.empty((5, m), dtype=np.float32)
    rhs[0:3] = -2.0 * b.T
    rhs[3] = 1.0
    rhs[4] = (b * b).sum(axis=1)
    return lhsT, rhs


def _split_cat(lhs, rhs):
    """fp32 [J, 5, n] operands -> K-stacked bf16 [J, 15, n] hi/lo forms."""
    lh = lhs.astype(_NP_BF16)
    ll = (lhs - lh.astype(np.float32)).astype(_NP_BF16)
    rh = rhs.astype(_NP_BF16)
    rl = (rhs - rh.astype(np.float32)).astype(_NP_BF16)
    lcat = np.concatenate([lh, lh, ll], axis=1)
    rcat = np.concatenate([rh, rl, rh], axis=1)
    # Replicate for the three PE row groups (partition offsets 0/32/64).
    lrep = np.repeat(lcat[:, None, :, :], 3, axis=1)
    rrep = np.repeat(rcat[:, None, :, :], 3, axis=1)
    return np.ascontiguousarray(lrep), np.ascontiguousarray(rrep)


def _in_maps(predictions, targets):
    in_maps = []
    for core in range(N_CORES):
        lhs = np.empty((JOBS, 5, N), dtype=np.float32)
        rhs = np.empty((JOBS, 5, M), dtype=np.float32)
        for bi in range(BPC):
            b = core * BPC + bi
            # direction 0: rows = predictions, min over targets
            lhs[2 * bi], rhs[2 * bi] = _augment(predictions[b], targets[b])
            # direction 1: rows = targets, min over predictions
            lhs[2 * bi + 1], rhs[2 * bi + 1] = _augment(targets[b], predictions[b])
        lcat, rcat = _split_cat(lhs, rhs)
        in_maps.append({"lhs": lcat, "rhs": rcat})
    return in_maps


def kernel(predictions, targets):
    predictions = np.asarray(predictions, dtype=np.float32)
    targets = np.asarray(targets, dtype=np.float32)

    nc = _get_nc()
    res = run_bass_kernel_spmd(
        nc, _in_maps(predictions, targets), core_ids=list(range(N_CORES))
    )

    total = 0.0
    for core in range(N_CORES):
        total -= res.results[core]["maxs"].astype(np.float64).sum()
    return np.float32(total)
